# revision 1
# baseline (speedup 1.0000x reference)
"""Trainium2 Bass kernel for nn_DecoderGravity (edge-list gravity decoder).

Computes, for each edge e with src s=idx[0,e], dst d=idx[1,e]:
    out[e] = x[d, 128] - l * log(sum_k (x[s,k]-x[d,k])^2 + 0.01)

Strategy (8 NeuronCores, 80000 edges each):
  * On device, each core repacks the node table x [50000,129] f32 into
    gatherable tables: XH (fp16 positions, 256B rows) and XM (f32 mass,
    256B rows), each split at row 32768 because dma_gather indices are
    int16. One-time cost ~40MB of DMA per core.
  * Edges are bucketed host-side (part of sharding) by (src>=32768,
    dst>=32768) into 4 buckets so every dma_gather uses half-local
    indices; buckets are padded to fixed capacities with dummy edges.
  * Steady state: per tile of 128*kc edges, three dma_gathers (src
    positions, dst positions, dst mass), fp16 subtract (DVE), fp16
    square (ScalarE), f32 reduce (DVE), then a single log/mul/add
    epilogue over the whole core's [128, 672] result.
  * r2 for random 128-dim gaussian pairs is >= ~150, so fp16 position
    precision gives ~4e-5 scale-relative output error.
"""

import numpy as np

import concourse.bass as bass
import concourse.tile as tile
from concourse import bacc, mybir
from concourse.bass_utils import run_bass_kernel_spmd

# Problem constants (hardcoded per contract).
N = 50000
D = 129
DM = 128
E = 640000
NUM_CORES = 8
P = 128
EC = E // NUM_CORES          # 80000 edges per core
HALF = 32768                 # int16-index table split point
NHI = 17280                  # hi-half rows (17232 real + pad)
EPS = 0.01

# bucket capacities in 128-edge columns: ll, lh, hl, hh
CAPC = (280, 152, 152, 88)   # 35840, 19456, 19456, 11264 edges
COLS = sum(CAPC)             # 672
KC_BIG = 16                  # columns per gather tile (2048 indices)

f32 = mybir.dt.float32
fp16 = mybir.dt.float16
i16 = mybir.dt.int16


def _tiles(cols, k):
    out = []
    while cols > 0:
        out.append(min(k, cols))
        cols -= out[-1]
    return out


def build_program(num_cores=NUM_CORES, capc=CAPC, kc=KC_BIG, bufs=2):
    cols = sum(capc)
    nc = bacc.Bacc("TRN2", target_bir_lowering=False, debug=False,
                   num_devices=num_cores)
    x_ap = nc.dram_tensor("x", [N, D], f32, kind="ExternalInput").ap()
    s16_ap = nc.dram_tensor("src16", [P, cols * 8], i16,
                            kind="ExternalInput").ap()
    d16_ap = nc.dram_tensor("dst16", [P, cols * 8], i16,
                            kind="ExternalInput").ap()
    l_ap = nc.dram_tensor("l_param", [1, 1], f32, kind="ExternalInput").ap()
    out_ap = nc.dram_tensor("out", [P, cols], f32, kind="ExternalOutput").ap()

    xh_lo = nc.dram_tensor("xh_lo", [HALF, DM], fp16).ap()
    xh_hi = nc.dram_tensor("xh_hi", [NHI, DM], fp16).ap()
    # dst table: 512B rows [x fp16(128) | mass f32 packed as 2 u16 | pad]
    xd_lo = nc.dram_tensor("xd_lo", [HALF, 256], fp16).ap()
    xd_hi = nc.dram_tensor("xd_hi", [NHI, 256], fp16).ap()

    with tile.TileContext(nc) as tc:
        with (
            tc.tile_pool(name="bld", bufs=2) as bldp,
            tc.tile_pool(name="idx", bufs=1) as idxp,
            tc.tile_pool(name="wide", bufs=1) as widep,
            tc.tile_pool(name="gsrc", bufs=bufs) as srcp,
            tc.tile_pool(name="gdst", bufs=bufs) as dstp,
            tc.tile_pool(name="sq", bufs=2) as sqp,
        ):
            # ---- table build: lo half, then hi half -------------------
            # lo: x rows [0, 32768) viewed [128, 256, 129]
            x_lo = x_ap[0:HALF].rearrange("(p c) d -> p c d", p=P)
            xh_lo_v = xh_lo[:].rearrange("(p c) d -> p c d", p=P)
            xd_lo_v = xd_lo[:].rearrange("(p c) d -> p c d", p=P)
            # hi main: x rows [32768, 49920) viewed [128, 134, 129]
            x_hi = x_ap[HALF:HALF + 128 * 134].rearrange(
                "(p c) d -> p c d", p=P)
            xh_hi_v = xh_hi[0:128 * 134].rearrange("(p c) d -> p c d", p=P)
            xd_hi_v = xd_hi[0:128 * 134].rearrange("(p c) d -> p c d", p=P)

            def build_chunk(xv, xhv, xdv, c0, ck):
                xt = bldp.tile([P, 32, D], f32, tag="bx")
                nc.sync.dma_start(xt[:, :ck, :], xv[:, c0:c0 + ck, :])
                dt = bldp.tile([P, 32, 130], fp16, tag="bd")
                nc.vector.tensor_copy(dt[:, :ck, 0:DM], xt[:, :ck, 0:DM])
                # mass f32 bits -> two u16 lanes at cols 128:130
                nc.vector.tensor_copy(
                    dt[:, :ck, DM:DM + 2].bitcast(mybir.dt.uint16),
                    xt[:, :ck, DM:D].bitcast(mybir.dt.uint16))
                nc.sync.dma_start(xhv[:, c0:c0 + ck, :], dt[:, :ck, 0:DM])
                nc.sync.dma_start(xdv[:, c0:c0 + ck, 0:130], dt[:, :ck, :])

            for c0 in range(0, 256, 32):
                build_chunk(x_lo, xh_lo_v, xd_lo_v, c0, 32)
            for c0 in range(0, 134, 32):
                build_chunk(x_hi, xh_hi_v, xd_hi_v, c0, min(32, 134 - c0))
            # hi tail: x rows [49920, 50000) -> table rows [17152, 17232)
            xt = bldp.tile([80, 1, D], f32, tag="btail")
            nc.sync.dma_start(xt[:], x_ap[49920:50000].unsqueeze(1))
            dt = bldp.tile([80, 1, 130], fp16, tag="btaild")
            nc.vector.tensor_copy(dt[:, :, 0:DM], xt[:, :, 0:DM])
            nc.vector.tensor_copy(
                dt[:, :, DM:DM + 2].bitcast(mybir.dt.uint16),
                xt[:, :, DM:D].bitcast(mybir.dt.uint16))
            nc.sync.dma_start(xh_hi[17152:17232].unsqueeze(1), dt[:, :, 0:DM])
            nc.sync.dma_start(xd_hi[17152:17232].unsqueeze(1)[:, :, 0:130],
                              dt[:])

            # ---- small setup -----------------------------------------
            s16_sb = idxp.tile([P, cols * 8], i16, tag="s16")
            d16_sb = idxp.tile([P, cols * 8], i16, tag="d16")
            nc.sync.dma_start(s16_sb[:], s16_ap[:])
            nc.sync.dma_start(d16_sb[:], d16_ap[:])

            lrow = widep.tile([1, 1], f32, tag="lrow")
            nc.sync.dma_start(lrow[:], l_ap[:])
            lbc = widep.tile([P, 1], f32, tag="lbc")
            nc.gpsimd.partition_broadcast(lbc[:], lrow[:], channels=P)
            lneg = widep.tile([P, 1], f32, tag="lneg")
            nc.vector.tensor_scalar_mul(lneg[:], lbc[:], -1.0)
            epsb = widep.tile([P, 1], f32, tag="eps")
            nc.gpsimd.memset(epsb[:], EPS)

            r2w = widep.tile([P, cols], f32, tag="r2")
            mw = widep.tile([P, cols], f32, tag="m")
            logw = widep.tile([P, cols], f32, tag="logw")
            outw = widep.tile([P, cols], f32, tag="outw")

            # ---- gather + compute loop -------------------------------
            # bucket b = (src_hi)*2 + (dst_hi); process ll first so its
            # gathers only wait on the lo tables.
            off = 0
            for b, bc in enumerate(capc):
                s_tab = xh_lo if b < 2 else xh_hi
                d_tab = xd_lo if b % 2 == 0 else xd_hi
                for ck in _tiles(bc, kc):
                    sl = slice(off, off + ck)
                    isl = slice(off * 8, (off + ck) * 8)
                    nidx = ck * P
                    src_t = srcp.tile([P, kc, DM], fp16, tag="srct")
                    nc.gpsimd.dma_gather(src_t[:, :ck, :], s_tab[:],
                                         s16_sb[:, isl], nidx, nidx, DM,
                                         single_packet=False)
                    dst_t = dstp.tile([P, kc, 256], fp16, tag="dstt")
                    nc.gpsimd.dma_gather(dst_t[:, :ck, :], d_tab[:],
                                         d16_sb[:, isl], nidx, nidx, 256,
                                         single_packet=False)
                    nc.vector.tensor_copy(
                        mw[:, sl].unsqueeze(2),
                        dst_t[:, :ck, DM:DM + 2].bitcast(f32))
                    nc.vector.tensor_tensor(
                        out=dst_t[:, :ck, 0:DM], in0=src_t[:, :ck, :],
                        in1=dst_t[:, :ck, 0:DM], op=mybir.AluOpType.subtract)
                    sq_t = sqp.tile([P, kc, DM], fp16, tag="sq")
                    nc.scalar.activation(sq_t[:, :ck, :],
                                         dst_t[:, :ck, 0:DM],
                                         mybir.ActivationFunctionType.Square)
                    nc.vector.tensor_reduce(r2w[:, sl], sq_t[:, :ck, :],
                                            axis=mybir.AxisListType.X,
                                            op=mybir.AluOpType.add)
                    off += ck

            nc.scalar.activation(logw[:], r2w[:],
                                 mybir.ActivationFunctionType.Ln,
                                 bias=epsb[:, 0:1])
            nc.vector.scalar_tensor_tensor(
                out=outw[:], in0=logw[:], scalar=lneg[:, 0:1], in1=mw[:],
                op0=mybir.AluOpType.mult, op1=mybir.AluOpType.add)
            nc.sync.dma_start(out_ap[:], outw[:])

    nc.compile()
    return nc


_compiled = {}


def _get_compiled(capc=CAPC):
    if capc not in _compiled:
        _compiled[capc] = build_program(capc=capc)
    return _compiled[capc]


def _wrap16(vals: np.ndarray, cap_edges: int) -> np.ndarray:
    """int16 index list -> [128, cap/16] wrapped+replicated layout."""
    arr = np.zeros(cap_edges, np.int16)
    arr[: len(vals)] = vals
    w = arr.reshape(cap_edges // 16, 16).T        # [16, cap/16]
    return np.tile(w, (8, 1))                     # [128, cap/16]


def make_in_maps(x, edge_label_index, l_param, capc=CAPC):
    x = np.ascontiguousarray(np.asarray(x, dtype=np.float32))
    eli = np.asarray(edge_label_index)
    l = np.asarray(l_param, dtype=np.float32).reshape(1, 1)
    src = eli[0].astype(np.int64)
    dst = eli[1].astype(np.int64)
    in_maps = []
    orders = []
    counts_all = []
    for c in range(NUM_CORES):
        sl = slice(c * EC, (c + 1) * EC)
        s, d = src[sl], dst[sl]
        b = (s >= HALF) * 2 + (d >= HALF)
        order = np.argsort(b, kind="stable")
        counts = np.bincount(b, minlength=4)
        if np.any(counts > np.array(capc) * P):
            raise OverflowError(list(counts))
        s_loc = (s - HALF * (s >= HALF)).astype(np.int16)
        d_loc = (d - HALF * (d >= HALF)).astype(np.int16)
        sw_parts, dw_parts = [], []
        pos = 0
        for bi in range(4):
            es = order[pos: pos + counts[bi]]
            pos += counts[bi]
            cap = capc[bi] * P
            sw_parts.append(_wrap16(s_loc[es], cap))
            dw_parts.append(_wrap16(d_loc[es], cap))
        in_maps.append({
            "x": x,
            "src16": np.ascontiguousarray(np.concatenate(sw_parts, axis=1)),
            "dst16": np.ascontiguousarray(np.concatenate(dw_parts, axis=1)),
            "l_param": l,
        })
        orders.append(order)
        counts_all.append(counts)
    return in_maps, orders, counts_all


def _unshard(results, orders, counts_all, capc=CAPC):
    out = np.empty(E, np.float32)
    offs = np.cumsum([0] + [c for c in capc])
    for c in range(NUM_CORES):
        dev = results[c]["out"]            # [128, cols]
        order, counts = orders[c], counts_all[c]
        core_out = np.empty(EC, np.float32)
        pos = 0
        for bi in range(4):
            cnt = counts[bi]
            vals = dev[:, offs[bi]: offs[bi] + capc[bi]].T.ravel()[:cnt]
            core_out[order[pos: pos + cnt]] = vals
            pos += cnt
        out[c * EC:(c + 1) * EC] = core_out
    return out.reshape(E, 1)


def kernel(x, edge_label_index, l_param):
    capc = CAPC
    while True:
        try:
            in_maps, orders, counts = make_in_maps(
                x, edge_label_index, l_param, capc)
            break
        except OverflowError as e:
            # grow capacities to fit (rounded up to tile granularity)
            need = [max(int(np.ceil(n / P / 8)) * 8, c)
                    for n, c in zip(e.args[0], capc)]
            capc = tuple(need)
    nc = _get_compiled(capc)
    res = run_bass_kernel_spmd(nc, in_maps, list(range(NUM_CORES)))
    return _unshard(res.results, orders, counts, capc)



# revision 3
# speedup vs baseline: 1.5872x; 1.5872x over previous
"""Trainium2 Bass kernel for nn_DecoderGravity (edge-list gravity decoder).

Computes, for each edge e with src s=idx[0,e], dst d=idx[1,e]:
    out[e] = x[d, 128] - l * log(sum_k (x[s,k]-x[d,k])^2 + 0.01)

Strategy (8 NeuronCores, 80000 edges each). The v1 kernel was bottlenecked
by GPSIMD SWDGE descriptor generation for dma_gather (~8ns/index, 160k
indices/core = 1.3ms). v2 halves+ that:

  * dst side: ONE dma_gather stream from a "pair table" xp[25088, 512B]
    (fp8 features + fp16 mass for nodes 2r and 2r+1 packed in one row,
    index = dst//2 fits int16 without lo/hi bucketing). 86016 padded
    slots -> ~690us of Pool time. Even/odd halves are blended at the r2
    level by a host-provided parity mask.
  * src side: NO dma_gather. Edges are sorted by src block (128 nodes)
    and packed into 128-edge chunks such that chunk c only draws from a
    static window of K=2 blocks W(c). A one-hot matrix (DVE is_equal vs
    iota) times the fp8 node table x_sb [128, 391*128] on the Tensor
    engine materializes gathered src rows into PSUM (edge-major).
  * r2 = reduce((s - d)^2): ACT copies PSUM->fp16, DVE subtract, ACT
    square, DVE reduce; done for even and odd dst halves, blended by
    parity. Epilogue: out = m - l*ln(r2 + eps).
  * fp8 e4m3 position quantization gives ~0.3% r2 error -> ~3e-3 abs
    output error, far inside the 2e-2 gate.
"""

import numpy as np

import concourse.bass as bass
import concourse.tile as tile
from concourse import bacc, mybir
from concourse.bass_utils import run_bass_kernel_spmd

# Problem constants (hardcoded per contract).
N = 50000
D = 129
DM = 128
E = 640000
NUM_CORES = 8
P = 128
EC = E // NUM_CORES          # 80000 edges per core
NPAD = 50176                 # N padded to 128*392
NPAIR = NPAD // 2            # pair-table rows
NB = 391                     # src blocks of 128 nodes covering 50048
K = 2                        # block window size per chunk
C_DEF = 672                  # chunks of 128 edge slots (pad >= 5%)
KC = 16                      # chunks per gather tile (2048 slots)
EPS = 0.01
PAD_SRCLOC = 1000.0          # matches no iota value -> zero one-hot col

f32 = mybir.dt.float32
fp16 = mybir.dt.float16
fp8 = mybir.dt.float8e4
i16 = mybir.dt.int16


def _w_lo(C):
    alpha = NB / C
    return np.minimum((np.arange(C) * alpha).astype(int), NB - K)


def build_program(C=C_DEF):
    assert C % KC == 0
    ntiles = C // KC
    w_lo = _w_lo(C)
    nc = bacc.Bacc("TRN2", target_bir_lowering=False, debug=False,
                   num_devices=NUM_CORES)
    x_ap = nc.dram_tensor("xpad", [NPAD, D], f32, kind="ExternalInput").ap()
    d16_ap = nc.dram_tensor("dst16", [P, C * 8], i16,
                            kind="ExternalInput").ap()
    sl_ap = nc.dram_tensor("srcloc", [1, C * P], fp16,
                           kind="ExternalInput").ap()
    par_ap = nc.dram_tensor("par", [P, C], fp16, kind="ExternalInput").ap()
    cst_ap = nc.dram_tensor("nl_eps", [P, 2], f32, kind="ExternalInput").ap()
    io_ap = nc.dram_tensor("iota2", [P, K], f32, kind="ExternalInput").ap()
    out_ap = nc.dram_tensor("out", [P, C], f32, kind="ExternalOutput").ap()

    xp = nc.dram_tensor("xp", [NPAIR, 512], fp8).ap()

    with tile.TileContext(nc) as tc:
        with (
            tc.tile_pool(name="xt", bufs=2) as xtp,
            tc.tile_pool(name="pair", bufs=2) as pairp,
            tc.tile_pool(name="xsb", bufs=1) as xsbp,
            tc.tile_pool(name="oh", bufs=2) as ohp,
            tc.tile_pool(name="srcb", bufs=2) as srcbp,
            tc.tile_pool(name="ssb", bufs=2) as ssbp,
            tc.tile_pool(name="dq", bufs=2) as dqp,
            tc.tile_pool(name="sq", bufs=2) as sqp,
            tc.tile_pool(name="wide", bufs=1) as widep,
            tc.tile_pool(name="ps", bufs=2, space="PSUM") as psp,
        ):
            # ---- phase A: build pair table xp ------------------------
            # x rows viewed as [p, 196 pairs, 2, 129]; xp as [p, 196, 512]
            xv = x_ap[:].rearrange("(p c two) d -> p c (two d)", p=P, two=2)
            xpv = xp[:].rearrange("(p c) d -> p c d", p=P)
            CP = NPAIR // P  # 196
            for c0 in range(0, CP, 16):
                cw = min(16, CP - c0)
                t = xtp.tile([P, 4128], f32, tag="xt")
                tv = t.rearrange("p (c d) -> p c d", d=2 * D)
                nc.sync.dma_start(tv[:, :cw, :], xv[:, c0:c0 + cw, :])
                dt = pairp.tile([P, KC, 512], fp8, tag="pair")
                dt16 = dt.bitcast(fp16)
                nc.vector.tensor_copy(dt[:, :cw, 0:DM], tv[:, :cw, 0:DM])
                nc.vector.tensor_copy(dt16[:, :cw, 64:65],
                                      tv[:, :cw, DM:DM + 1])
                nc.vector.tensor_copy(dt[:, :cw, 256:256 + DM],
                                      tv[:, :cw, D:D + DM])
                nc.vector.tensor_copy(dt16[:, :cw, 192:193],
                                      tv[:, :cw, 2 * D - 1:2 * D])
                nc.sync.dma_start(xpv[:, c0:c0 + cw, :], dt[:, :cw, :])

            # ---- phase B: x_sb fp8 node table (partition = node%128) --
            xv2 = x_ap[:].rearrange("(c p) d -> p c d", p=P)
            x_sb = xsbp.tile([P, NB * DM], fp8, tag="xsb")
            xsv = x_sb.rearrange("p (c f) -> p c f", f=DM)
            for c0 in range(0, NB, 32):
                cw = min(32, NB - c0)
                t = xtp.tile([P, 4128], f32, tag="xt")
                tv = t.rearrange("p (c d) -> p c d", d=D)
                nc.sync.dma_start(tv[:, :cw, :], xv2[:, c0:c0 + cw, :])
                nc.vector.tensor_copy(xsv[:, c0:c0 + cw, :], tv[:, :cw, 0:DM])

            # ---- phase C: small loads --------------------------------
            idx_sb = widep.tile([P, C * 8], i16, tag="idx")
            nc.sync.dma_start(idx_sb[:], d16_ap[:])
            par_sb = widep.tile([P, C], fp16, tag="par")
            nc.sync.dma_start(par_sb[:], par_ap[:])
            cst = widep.tile([P, 2], f32, tag="cst")
            nc.sync.dma_start(cst[:], cst_ap[:])
            iot = widep.tile([P, K], f32, tag="iota")
            nc.sync.dma_start(iot[:], io_ap[:])

            r2e_w = widep.tile([P, C], f32, tag="r2e")
            r2o_w = widep.tile([P, C], f32, tag="r2o")
            me_w = widep.tile([P, C], f32, tag="me")
            mo_w = widep.tile([P, C], f32, tag="mo")
            outw = widep.tile([P, C], f32, tag="outw")

            # ---- phase D: main loop ----------------------------------
            for t_i in range(ntiles):
                sl2 = slice(t_i * KC * P, (t_i + 1) * KC * P)
                slc = slice(t_i * KC, (t_i + 1) * KC)
                isl = slice(t_i * P, (t_i + 1) * P)
                srcb = srcbp.tile([P, KC * P], fp16, tag="srcb")
                nc.sync.dma_start(
                    srcb[:], sl_ap[0:1, sl2].partition_broadcast(P))
                oh = ohp.tile([P, K, KC * P], fp8, tag="oh")
                for k in range(K):
                    nc.vector.tensor_scalar(
                        oh[:, k:k + 1, :], srcb.unsqueeze(1),
                        iot[:, k:k + 1], None, op0=mybir.AluOpType.is_equal)
                ps = psp.tile([P, KC * P], f32, tag="ps")
                for ch in range(KC):
                    g = t_i * KC + ch
                    w = int(w_lo[g])
                    cs = slice(ch * P, (ch + 1) * P)
                    for k in range(K):
                        nc.tensor.matmul(
                            ps[:, cs], oh[:, k:k + 1, cs],
                            xsv[:, w + k, :],
                            start=(k == 0), stop=(k == K - 1))
                pt = pairp.tile([P, KC, 512], fp8, tag="pair")
                nc.gpsimd.dma_gather(pt[:], xp[:], idx_sb[:, isl],
                                     KC * P, KC * P, 512,
                                     single_packet=False)
                pt16 = pt.bitcast(fp16)
                ssb = ssbp.tile([P, KC * P], fp16, tag="ssb")
                nc.scalar.activation(ssb[:], ps[:],
                                     mybir.ActivationFunctionType.Copy)
                sv = ssb.rearrange("p (c f) -> p c f", f=DM)
                de = dqp.tile([P, KC, DM], fp16, tag="dq")
                nc.vector.tensor_tensor(out=de[:], in0=sv[:],
                                        in1=pt[:, :, 0:DM],
                                        op=mybir.AluOpType.subtract)
                se = sqp.tile([P, KC, DM], fp16, tag="sq")
                nc.scalar.activation(se[:], de[:],
                                     mybir.ActivationFunctionType.Square)
                nc.vector.tensor_reduce(r2e_w[:, slc], se[:],
                                        axis=mybir.AxisListType.X,
                                        op=mybir.AluOpType.add)
                do = dqp.tile([P, KC, DM], fp16, tag="dq")
                nc.vector.tensor_tensor(out=do[:], in0=sv[:],
                                        in1=pt[:, :, 256:256 + DM],
                                        op=mybir.AluOpType.subtract)
                so = sqp.tile([P, KC, DM], fp16, tag="sq")
                nc.scalar.activation(so[:], do[:],
                                     mybir.ActivationFunctionType.Square)
                nc.vector.tensor_reduce(r2o_w[:, slc], so[:],
                                        axis=mybir.AxisListType.X,
                                        op=mybir.AluOpType.add)
                nc.vector.tensor_copy(me_w[:, slc].unsqueeze(2),
                                      pt16[:, :, 64:65])
                nc.vector.tensor_copy(mo_w[:, slc].unsqueeze(2),
                                      pt16[:, :, 192:193])

            # ---- phase E: epilogue -----------------------------------
            # r2 = r2e + par*(r2o - r2e); m likewise; out = m - l*ln(r2+eps)
            nc.vector.tensor_tensor(out=r2o_w[:], in0=r2o_w[:], in1=r2e_w[:],
                                    op=mybir.AluOpType.subtract)
            nc.vector.tensor_tensor(out=r2o_w[:], in0=r2o_w[:], in1=par_sb[:],
                                    op=mybir.AluOpType.mult)
            nc.vector.tensor_tensor(out=r2e_w[:], in0=r2e_w[:], in1=r2o_w[:],
                                    op=mybir.AluOpType.add)
            nc.vector.tensor_tensor(out=mo_w[:], in0=mo_w[:], in1=me_w[:],
                                    op=mybir.AluOpType.subtract)
            nc.vector.tensor_tensor(out=mo_w[:], in0=mo_w[:], in1=par_sb[:],
                                    op=mybir.AluOpType.mult)
            nc.vector.tensor_tensor(out=me_w[:], in0=me_w[:], in1=mo_w[:],
                                    op=mybir.AluOpType.add)
            nc.scalar.activation(r2o_w[:], r2e_w[:],
                                 mybir.ActivationFunctionType.Ln,
                                 bias=cst[:, 1:2])
            nc.vector.scalar_tensor_tensor(
                out=outw[:], in0=r2o_w[:], scalar=cst[:, 0:1], in1=me_w[:],
                op0=mybir.AluOpType.mult, op1=mybir.AluOpType.add)
            nc.sync.dma_start(out_ap[:], outw[:])

    nc.compile()
    return nc


_compiled = {}


def _get_compiled(C=C_DEF):
    if C not in _compiled:
        _compiled[C] = build_program(C)
    return _compiled[C]


def _pack_core(src, dst, C):
    """Window-pack edges (sorted by src block) into C chunks of 128 slots.

    Returns slot2edge [C*128] int64 (-1 = pad). Raises OverflowError if C
    is too small.
    """
    w_lo = _w_lo(C)
    order = np.argsort(src, kind="stable")
    blocks = (src[order] // P).astype(np.int64)
    counts = np.bincount(blocks, minlength=NB)
    slot2edge = np.full(C * P, -1, np.int64)
    c = 0
    fill = 0
    pos = 0
    for b in range(NB):
        n = int(counts[b])
        while n > 0:
            while c < C and not (w_lo[c] <= b < w_lo[c] + K):
                c += 1
                fill = 0
            if c >= C:
                raise OverflowError(C)
            take = min(n, P - fill)
            s0 = c * P + fill
            slot2edge[s0:s0 + take] = order[pos:pos + take]
            pos += take
            n -= take
            fill += take
            if fill == P:
                c += 1
                fill = 0
                if c >= C and n > 0:
                    raise OverflowError(C)
    return slot2edge


def make_in_maps(x, edge_label_index, l_param, C=C_DEF):
    x = np.asarray(x, dtype=np.float32)
    x_pad = np.zeros((NPAD, D), np.float32)
    x_pad[:N] = x
    eli = np.asarray(edge_label_index)
    l = float(np.asarray(l_param).reshape(-1)[0])
    nl_eps = np.zeros((P, 2), np.float32)
    nl_eps[:, 0] = -l
    nl_eps[:, 1] = EPS
    iota2 = np.zeros((P, K), np.float32)
    for k in range(K):
        iota2[:, k] = k * P + np.arange(P)
    w_lo = _w_lo(C)
    chunk_of_slot = np.arange(C * P) // P

    in_maps = []
    slot_maps = []
    for core in range(NUM_CORES):
        sl = slice(core * EC, (core + 1) * EC)
        src = eli[0][sl].astype(np.int64)
        dst = eli[1][sl].astype(np.int64)
        s2e = _pack_core(src, dst, C)
        real = s2e >= 0
        e = s2e[real]
        srcloc = np.full(C * P, PAD_SRCLOC, np.float16)
        srcloc[real] = (src[e] - P * w_lo[chunk_of_slot[real]]).astype(
            np.float16)
        dstv = np.zeros(C * P, np.int16)
        dstv[real] = (dst[e] // 2).astype(np.int16)
        d16 = np.tile(dstv.reshape(C * P // 16, 16).T, (8, 1))
        par = np.zeros(C * P, np.float16)
        par[real] = (dst[e] % 2).astype(np.float16)
        in_maps.append({
            "xpad": x_pad,
            "dst16": np.ascontiguousarray(d16),
            "srcloc": np.ascontiguousarray(srcloc.reshape(1, C * P)),
            "par": np.ascontiguousarray(par.reshape(C, P).T),
            "nl_eps": nl_eps,
            "iota2": iota2,
        })
        slot_maps.append(s2e)
    return in_maps, slot_maps, C


def _unshard(results, slot_maps, C):
    out = np.empty(E, np.float32)
    for core in range(NUM_CORES):
        dev = results[core]["out"]          # [128, C]
        vals = dev.T.ravel()                # slot-major
        s2e = slot_maps[core]
        real = s2e >= 0
        core_out = np.empty(EC, np.float32)
        core_out[s2e[real]] = vals[real]
        out[core * EC:(core + 1) * EC] = core_out
    return out.reshape(E, 1)


def kernel(x, edge_label_index, l_param):
    C = C_DEF
    while True:
        try:
            in_maps, slot_maps, C = make_in_maps(
                x, edge_label_index, l_param, C)
            break
        except OverflowError:
            C += 2 * KC
    nc = _get_compiled(C)
    res = run_bass_kernel_spmd(nc, in_maps, list(range(NUM_CORES)))
    return _unshard(res.results, slot_maps, C)


# revision 4
# speedup vs baseline: 1.8423x; 1.1608x over previous
"""Trainium2 Bass kernel for nn_DecoderGravity (edge-list gravity decoder).

Computes, for each edge e with src s=idx[0,e], dst d=idx[1,e]:
    out[e] = x[d, 128] - l * log(sum_k (x[s,k]-x[d,k])^2 + 0.01)

The v1 kernel was bottlenecked by GPSIMD SWDGE descriptor generation for
dma_gather (~8ns/index, 160k indices/core = 1.3ms). This version halves
the descriptor stream and overlaps everything else under it:

  * dst side: ONE dma_gather stream from a "pair table" xp[25088, 512B]
    (fp8 features + fp16 mass for nodes 2r and 2r+1 in one row; index =
    dst//2 fits int16 without lo/hi bucketing). 86016 padded slots ->
    ~690us of Pool time, the kernel's roofline. Even/odd halves are
    blended at the r2/mass level with a host parity mask.
  * src side: NO dma_gather. Edges are sorted by src block (128 nodes)
    and packed into 128-edge chunks such that chunk c only draws from a
    static window of K=2 blocks. Host ships per-chunk one-hot planes
    (fp8; pure index metadata); the Tensor engine multiplies them with
    the fp8 node table x_sb [128, 391*128] to materialize gathered src
    rows in PSUM (edge-major), overlapped under the Pool roof.
  * r2 = reduce((s-d)^2): ACT copies PSUM->fp16, DVE subtracts, ACT
    squares, DVE reduces (fp16 accum; ~1% r2 error is far inside the
    gate) for both dst halves. Epilogue: out = m - l*ln(r2 + eps).
  * Emission is software-pipelined: produce(t) [oh DMA, matmuls,
    gather] is emitted before consume(t-1) so no engine head-of-line
    blocks the Pool descriptor stream.
"""

import numpy as np
import ml_dtypes

import concourse.bass as bass
import concourse.tile as tile
from concourse import bacc, mybir
from concourse.bass_utils import run_bass_kernel_spmd

# Problem constants (hardcoded per contract).
N = 50000
D = 129
DM = 128
E = 640000
NUM_CORES = 8
P = 128
EC = E // NUM_CORES          # 80000 edges per core
NPAD = 50176                 # N padded to 128*392
NPAIR = NPAD // 2            # pair-table rows
NB = 391                     # src blocks of 128 nodes covering 50048
K = 2                        # block window size per chunk
C_DEF = 672                  # chunks of 128 edge slots (pad >= 5%)
KC = 16                      # chunks per gather tile (2048 slots)
TW = KC * P                  # slots per tile
EPS = 0.01

f32 = mybir.dt.float32
fp16 = mybir.dt.float16
fp8 = mybir.dt.float8e4
i16 = mybir.dt.int16
FP8NP = ml_dtypes.float8_e4m3


def _w_lo(C):
    alpha = NB / C
    return np.minimum((np.arange(C) * alpha).astype(int), NB - K)


def build_program(C=C_DEF):
    assert C % KC == 0
    ntiles = C // KC
    w_lo = _w_lo(C)
    nc = bacc.Bacc("TRN2", target_bir_lowering=False, debug=False,
                   num_devices=NUM_CORES)
    x_ap = nc.dram_tensor("xpad", [NPAD, D], f32, kind="ExternalInput").ap()
    d16_ap = nc.dram_tensor("dst16", [P, C * 8], i16,
                            kind="ExternalInput").ap()
    oh_ap = nc.dram_tensor("ohd", [P, ntiles * K * TW], fp8,
                           kind="ExternalInput").ap()
    par_ap = nc.dram_tensor("par", [P, C], fp16, kind="ExternalInput").ap()
    cst_ap = nc.dram_tensor("cst", [P, 2], fp16, kind="ExternalInput").ap()
    out_ap = nc.dram_tensor("out", [P, C], f32, kind="ExternalOutput").ap()

    xp = nc.dram_tensor("xp", [NPAIR, 512], fp8).ap()

    with tile.TileContext(nc) as tc:
        with (
            tc.tile_pool(name="xt", bufs=2) as xtp,
            tc.tile_pool(name="pair", bufs=3) as pairp,
            tc.tile_pool(name="xsb", bufs=1) as xsbp,
            tc.tile_pool(name="oh", bufs=2) as ohp,
            tc.tile_pool(name="ssb", bufs=2) as ssbp,
            tc.tile_pool(name="dq", bufs=2) as dqp,
            tc.tile_pool(name="sq", bufs=2) as sqp,
            tc.tile_pool(name="wide", bufs=1) as widep,
            tc.tile_pool(name="ps", bufs=2, space="PSUM") as psp,
        ):
            # ---- phase C: small loads (first: gathers dep on idx) ----
            idx_sb = widep.tile([P, C * 8], i16, tag="idx")
            nc.sync.dma_start(idx_sb[:], d16_ap[:])
            par_sb = widep.tile([P, C], fp16, tag="par")
            nc.sync.dma_start(par_sb[:], par_ap[:])
            cst = widep.tile([P, 2], fp16, tag="cst")
            nc.sync.dma_start(cst[:], cst_ap[:])

            r2e_w = widep.tile([P, C], fp16, tag="r2e")
            r2o_w = widep.tile([P, C], fp16, tag="r2o")
            m2_w = widep.tile([P, C * 2], fp16, tag="m2")
            tm_w = widep.tile([P, C], fp16, tag="tm")
            mm_w = widep.tile([P, C], fp16, tag="mm")
            outw = widep.tile([P, C], f32, tag="outw")
            m2v = m2_w.rearrange("p (c two) -> p c two", two=2)

            # ---- phase A: build pair table xp ------------------------
            # x rows viewed as [p, 196 pairs, 2, 129]; xp as [p, 196, 512]
            xv = x_ap[:].rearrange("(p c two) d -> p c (two d)", p=P, two=2)
            xpv = xp[:].rearrange("(p c) d -> p c d", p=P)
            CP = NPAIR // P  # 196
            for c0 in range(0, CP, 16):
                cw = min(16, CP - c0)
                t = xtp.tile([P, 4128], f32, tag="xt")
                tv = t.rearrange("p (c d) -> p c d", d=2 * D)
                nc.sync.dma_start(tv[:, :cw, :], xv[:, c0:c0 + cw, :])
                dt = pairp.tile([P, KC, 512], fp8, tag="pair")
                dt16 = dt.bitcast(fp16)
                nc.vector.tensor_copy(dt[:, :cw, 0:DM], tv[:, :cw, 0:DM])
                nc.vector.tensor_copy(dt16[:, :cw, 64:65],
                                      tv[:, :cw, DM:DM + 1])
                nc.vector.tensor_copy(dt[:, :cw, 256:256 + DM],
                                      tv[:, :cw, D:D + DM])
                nc.vector.tensor_copy(dt16[:, :cw, 192:193],
                                      tv[:, :cw, 2 * D - 1:2 * D])
                nc.sync.dma_start(xpv[:, c0:c0 + cw, :], dt[:, :cw, :])

            # ---- phase B: x_sb fp8 node table (partition = node%128) --
            xv2 = x_ap[:].rearrange("(c p) d -> p c d", p=P)
            x_sb = xsbp.tile([P, NB * DM], fp8, tag="xsb")
            xsv = x_sb.rearrange("p (c f) -> p c f", f=DM)
            for c0 in range(0, NB, 32):
                cw = min(32, NB - c0)
                t = xtp.tile([P, 4128], f32, tag="xt")
                tv = t.rearrange("p (c d) -> p c d", d=D)
                nc.sync.dma_start(tv[:, :cw, :], xv2[:, c0:c0 + cw, :])
                nc.vector.tensor_copy(xsv[:, c0:c0 + cw, :], tv[:, :cw, 0:DM])

            # ---- phase D: software-pipelined main loop ---------------
            tiles = {}

            def produce(t_i):
                isl = slice(t_i * P, (t_i + 1) * P)
                osl = slice(t_i * K * TW, (t_i + 1) * K * TW)
                oh = ohp.tile([P, K, TW], fp8, tag="oh")
                nc.sync.dma_start(oh.rearrange("p k w -> p (k w)"),
                                  oh_ap[:, osl])
                ps = psp.tile([P, TW], f32, tag="ps")
                for ch in range(KC):
                    g = t_i * KC + ch
                    w = int(w_lo[g])
                    cs = slice(ch * P, (ch + 1) * P)
                    for k in range(K):
                        nc.tensor.matmul(
                            ps[:, cs], oh[:, k:k + 1, cs],
                            xsv[:, w + k, :],
                            start=(k == 0), stop=(k == K - 1))
                pt = pairp.tile([P, KC, 512], fp8, tag="pair")
                nc.gpsimd.dma_gather(pt[:], xp[:], idx_sb[:, isl],
                                     TW, TW, 512, single_packet=False)
                tiles[t_i] = (ps, pt)

            def consume(t_i):
                ps, pt = tiles.pop(t_i)
                slc = slice(t_i * KC, (t_i + 1) * KC)
                sl2 = slice(t_i * KC * 2, (t_i + 1) * KC * 2)
                pt16 = pt.bitcast(fp16)
                ssb = ssbp.tile([P, TW], fp16, tag="ssb")
                nc.scalar.activation(ssb[:], ps[:],
                                     mybir.ActivationFunctionType.Copy)
                sv = ssb.rearrange("p (c f) -> p c f", f=DM)
                de = dqp.tile([P, KC, DM], fp16, tag="dq")
                nc.vector.tensor_tensor(out=de[:], in0=sv[:],
                                        in1=pt[:, :, 0:DM],
                                        op=mybir.AluOpType.subtract)
                se = sqp.tile([P, KC, DM], fp16, tag="sq")
                nc.scalar.activation(se[:], de[:],
                                     mybir.ActivationFunctionType.Square)
                with nc.allow_low_precision("r2 fp16 accum: ~1% worst-case"):
                    nc.vector.tensor_reduce(r2e_w[:, slc], se[:],
                                            axis=mybir.AxisListType.X,
                                            op=mybir.AluOpType.add)
                do = dqp.tile([P, KC, DM], fp16, tag="dq")
                nc.vector.tensor_tensor(out=do[:], in0=sv[:],
                                        in1=pt[:, :, 256:256 + DM],
                                        op=mybir.AluOpType.subtract)
                so = sqp.tile([P, KC, DM], fp16, tag="sq")
                nc.scalar.activation(so[:], do[:],
                                     mybir.ActivationFunctionType.Square)
                with nc.allow_low_precision("r2 fp16 accum: ~1% worst-case"):
                    nc.vector.tensor_reduce(r2o_w[:, slc], so[:],
                                            axis=mybir.AxisListType.X,
                                            op=mybir.AluOpType.add)
                nc.vector.tensor_copy(m2_w[:, sl2].rearrange(
                    "p (c two) -> p c two", two=2), pt16[:, :, 64:193:128])

            ntl = ntiles
            for t_i in range(ntl + 1):
                if t_i < ntl:
                    produce(t_i)
                if t_i > 0:
                    consume(t_i - 1)

            # ---- phase E: epilogue -----------------------------------
            # r2 = r2e + par*(r2o - r2e); m likewise; out = m - l*ln(r2+eps)
            nc.vector.tensor_tensor(out=r2o_w[:], in0=r2o_w[:], in1=r2e_w[:],
                                    op=mybir.AluOpType.subtract)
            nc.vector.tensor_tensor(out=r2o_w[:], in0=r2o_w[:], in1=par_sb[:],
                                    op=mybir.AluOpType.mult)
            nc.vector.tensor_tensor(out=r2e_w[:], in0=r2e_w[:], in1=r2o_w[:],
                                    op=mybir.AluOpType.add)
            nc.vector.tensor_tensor(out=tm_w.unsqueeze(2), in0=m2v[:, :, 1:2],
                                    in1=m2v[:, :, 0:1],
                                    op=mybir.AluOpType.subtract)
            nc.vector.tensor_tensor(out=tm_w[:], in0=tm_w[:], in1=par_sb[:],
                                    op=mybir.AluOpType.mult)
            nc.vector.tensor_tensor(out=mm_w.unsqueeze(2), in0=m2v[:, :, 0:1],
                                    in1=tm_w.unsqueeze(2),
                                    op=mybir.AluOpType.add)
            nc.scalar.activation(r2o_w[:], r2e_w[:],
                                 mybir.ActivationFunctionType.Ln,
                                 bias=cst[:, 1:2])
            nc.vector.scalar_tensor_tensor(
                out=outw[:], in0=r2o_w[:], scalar=cst[:, 0:1], in1=mm_w[:],
                op0=mybir.AluOpType.mult, op1=mybir.AluOpType.add)
            nc.sync.dma_start(out_ap[:], outw[:])

    nc.compile()
    return nc


_compiled = {}


def _get_compiled(C=C_DEF):
    if C not in _compiled:
        _compiled[C] = build_program(C)
    return _compiled[C]


def _pack_core(src, C):
    """Window-pack edges (sorted by src block) into C chunks of 128 slots.

    Returns slot2edge [C*128] int64 (-1 = pad). Raises OverflowError if C
    is too small.
    """
    w_lo = _w_lo(C)
    order = np.argsort(src, kind="stable")
    blocks = (src[order] // P).astype(np.int64)
    counts = np.bincount(blocks, minlength=NB)
    slot2edge = np.full(C * P, -1, np.int64)
    c = 0
    fill = 0
    pos = 0
    for b in range(NB):
        n = int(counts[b])
        while n > 0:
            while c < C and not (w_lo[c] <= b < w_lo[c] + K):
                c += 1
                fill = 0
            if c >= C:
                raise OverflowError(C)
            take = min(n, P - fill)
            s0 = c * P + fill
            slot2edge[s0:s0 + take] = order[pos:pos + take]
            pos += take
            n -= take
            fill += take
            if fill == P:
                c += 1
                fill = 0
                if c >= C and n > 0:
                    raise OverflowError(C)
    return slot2edge


def make_in_maps(x, edge_label_index, l_param, C=C_DEF):
    x = np.asarray(x, dtype=np.float32)
    x_pad = np.zeros((NPAD, D), np.float32)
    x_pad[:N] = x
    eli = np.asarray(edge_label_index)
    l = float(np.asarray(l_param).reshape(-1)[0])
    cstv = np.zeros((P, 2), np.float16)
    cstv[:, 0] = -l
    cstv[:, 1] = EPS
    w_lo = _w_lo(C)
    ntiles = C // KC
    chunk_of_slot = np.arange(C * P) // P

    in_maps = []
    slot_maps = []
    for core in range(NUM_CORES):
        sl = slice(core * EC, (core + 1) * EC)
        src = eli[0][sl].astype(np.int64)
        dst = eli[1][sl].astype(np.int64)
        s2e = _pack_core(src, C)
        real = s2e >= 0
        e = s2e[real]
        slots = np.arange(C * P)[real]
        v = src[e] - P * w_lo[chunk_of_slot[real]]   # [0, K*128)
        oh8 = np.zeros((P, ntiles, K, TW), np.uint8)
        oh8[v % P, slots // TW, v // P, slots % TW] = 0x38  # 1.0 in e4m3
        dstv = np.zeros(C * P, np.int16)
        dstv[real] = (dst[e] // 2).astype(np.int16)
        d16 = np.tile(dstv.reshape(C * P // 16, 16).T, (8, 1))
        par = np.zeros(C * P, np.float16)
        par[real] = (dst[e] % 2).astype(np.float16)
        in_maps.append({
            "xpad": x_pad,
            "dst16": np.ascontiguousarray(d16),
            "ohd": oh8.reshape(P, ntiles * K * TW).view(FP8NP),
            "par": np.ascontiguousarray(par.reshape(C, P).T),
            "cst": cstv,
        })
        slot_maps.append(s2e)
    return in_maps, slot_maps, C


def _unshard(results, slot_maps, C):
    out = np.empty(E, np.float32)
    for core in range(NUM_CORES):
        dev = results[core]["out"]          # [128, C]
        vals = dev.T.ravel()                # slot-major
        s2e = slot_maps[core]
        real = s2e >= 0
        core_out = np.empty(EC, np.float32)
        core_out[s2e[real]] = vals[real]
        out[core * EC:(core + 1) * EC] = core_out
    return out.reshape(E, 1)


def kernel(x, edge_label_index, l_param):
    C = C_DEF
    while True:
        try:
            in_maps, slot_maps, C = make_in_maps(
                x, edge_label_index, l_param, C)
            break
        except OverflowError:
            C += 2 * KC
    nc = _get_compiled(C)
    res = run_bass_kernel_spmd(nc, in_maps, list(range(NUM_CORES)))
    return _unshard(res.results, slot_maps, C)


# revision 10
# speedup vs baseline: 1.9268x; 1.0459x over previous
"""Trainium2 Bass kernel for nn_DecoderGravity (edge-list gravity decoder).

Computes, for each edge e with src s=idx[0,e], dst d=idx[1,e]:
    out[e] = x[d, 128] - l * log(sum_k (x[s,k]-x[d,k])^2 + 0.01)

The v1 kernel was bottlenecked by GPSIMD SWDGE descriptor generation for
dma_gather (~8ns/index, 160k indices/core = 1.3ms). This version halves
the descriptor stream and overlaps everything else under it:

  * dst side: ONE dma_gather stream from a "pair table" xp[25088, 512B]
    (fp8 features + fp16 mass for nodes 2r and 2r+1 in one row; index =
    dst//2 fits int16 without lo/hi bucketing). 86016 padded slots ->
    ~690us of Pool time, the kernel's roofline. Even/odd halves are
    blended at the r2/mass level with a host parity mask.
  * src side: NO dma_gather. Edges are sorted by src block (128 nodes)
    and packed into 128-edge chunks such that chunk c only draws from a
    static window of K=2 blocks. Host ships per-chunk one-hot planes
    (fp8; pure index metadata); the Tensor engine multiplies them with
    the fp8 node table x_sb [128, 391*128] to materialize gathered src
    rows in PSUM (edge-major), overlapped under the Pool roof.
  * r2 = reduce((s-d)^2): ACT copies PSUM->fp16, DVE subtracts, ACT
    squares, DVE reduces (fp16 accum; ~1% r2 error is far inside the
    gate) for both dst halves. Epilogue: out = m - l*ln(r2 + eps).
  * Emission is software-pipelined: produce(t) [oh DMA, matmuls,
    gather] is emitted before consume(t-1) so no engine head-of-line
    blocks the Pool descriptor stream.
"""

import numpy as np
import ml_dtypes

import concourse.bass as bass
import concourse.tile as tile
from concourse import bacc, mybir
from concourse.bass_utils import run_bass_kernel_spmd

# Problem constants (hardcoded per contract).
N = 50000
D = 129
DM = 128
E = 640000
NUM_CORES = 8
P = 128
EC = E // NUM_CORES          # 80000 edges per core
NPAD = 50176                 # N padded to 128*392
NPAIR = NPAD // 2            # pair-table rows
NB = 392                     # src blocks: node n in block n%392, row n//392
K = 2                        # block window size per chunk
C_DEF = 672                  # chunks of 128 edge slots (pad >= 5%)
KC = 16                      # chunks per gather tile (2048 slots)
TW = KC * P                  # slots per tile
EPS = 0.01

f32 = mybir.dt.float32
fp16 = mybir.dt.float16
fp8 = mybir.dt.float8e4
i16 = mybir.dt.int16
FP8NP = ml_dtypes.float8_e4m3


def _w_lo(C):
    alpha = NB / C
    return np.minimum((np.arange(C) * alpha).astype(int), NB - K)


def build_program(C=C_DEF):
    assert C % KC == 0
    ntiles = C // KC
    w_lo = _w_lo(C)
    nc = bacc.Bacc("TRN2", target_bir_lowering=False, debug=False,
                   num_devices=NUM_CORES)
    x_ap = nc.dram_tensor("xpad", [NPAD, D], f32, kind="ExternalInput").ap()
    d16_ap = nc.dram_tensor("dst16", [P, C * 8], i16,
                            kind="ExternalInput").ap()
    oh_ap = nc.dram_tensor("ohd", [P, ntiles * K * TW], fp8,
                           kind="ExternalInput").ap()
    par_ap = nc.dram_tensor("par", [P, C], fp16, kind="ExternalInput").ap()
    cst_ap = nc.dram_tensor("cst", [P, 2], fp16, kind="ExternalInput").ap()
    out_ap = nc.dram_tensor("out", [P, C], f32, kind="ExternalOutput").ap()

    xp = nc.dram_tensor("xp", [NPAIR, 512], fp8).ap()

    with tile.TileContext(nc) as tc:
        with (
            tc.tile_pool(name="xt", bufs=2) as xtp,
            tc.tile_pool(name="pair", bufs=4) as pairp,
            tc.tile_pool(name="xsb", bufs=1) as xsbp,
            tc.tile_pool(name="oh", bufs=2) as ohp,
            tc.tile_pool(name="ssb", bufs=2) as ssbp,
            tc.tile_pool(name="dq", bufs=2) as dqp,
            tc.tile_pool(name="sq", bufs=2) as sqp,
            tc.tile_pool(name="wide", bufs=1) as widep,
            tc.tile_pool(name="ps", bufs=2, space="PSUM") as psp,
        ):
            # ---- phase C: small loads (first: gathers dep on idx) ----
            idx_sb = widep.tile([P, C * 8], i16, tag="idx")
            nc.sync.dma_start(idx_sb[:], d16_ap[:])
            par_sb = widep.tile([P, C], fp16, tag="par")
            nc.sync.dma_start(par_sb[:], par_ap[:])
            cst = widep.tile([P, 2], fp16, tag="cst")
            nc.sync.dma_start(cst[:], cst_ap[:])

            r2e_w = widep.tile([P, C], fp16, tag="r2e")
            r2o_w = widep.tile([P, C], fp16, tag="r2o")
            m2_w = widep.tile([P, C * 2], fp16, tag="m2")
            tm_w = widep.tile([P, C], fp16, tag="tm")
            mm_w = widep.tile([P, C], fp16, tag="mm")
            outw = widep.tile([P, C], f32, tag="outw")
            m2v = m2_w.rearrange("p (c two) -> p c two", two=2)

            # ---- phase A: build pair table xp + x_sb in one x pass ----
            # x rows viewed as [p, 196 pairs, 2, 129]; xp as [p, 196, 512].
            # Node n = p*392 + c2 (c2 = 2c+t): src block = n%392 = c2,
            # position in block = n//392 = p. So the same tiles also fill
            # x_sb (strided over c2 parity) with no second x read.
            xv = x_ap[:].rearrange("(p c two) d -> p c (two d)", p=P, two=2)
            xpv = xp[:].rearrange("(p c) d -> p c d", p=P)
            x_sb = xsbp.tile([P, NB * DM], fp8, tag="xsb")
            xsv = x_sb.rearrange("p (c f) -> p c f", f=DM)
            CP = NPAIR // P  # 196
            for c0 in range(0, CP, 16):
                cw = min(16, CP - c0)
                t = xtp.tile([P, 4128], f32, tag="xt")
                tv = t.rearrange("p (c d) -> p c d", d=2 * D)
                nc.sync.dma_start(tv[:, :cw, :], xv[:, c0:c0 + cw, :])
                dt = pairp.tile([P, KC, 512], fp8, tag="pair")
                dt16 = dt.bitcast(fp16)
                nc.vector.tensor_copy(dt[:, :cw, 0:DM], tv[:, :cw, 0:DM])
                nc.vector.tensor_copy(dt16[:, :cw, 64:65],
                                      tv[:, :cw, DM:DM + 1])
                nc.vector.tensor_copy(dt[:, :cw, 256:256 + DM],
                                      tv[:, :cw, D:D + DM])
                nc.vector.tensor_copy(dt16[:, :cw, 65:66],
                                      tv[:, :cw, 2 * D - 1:2 * D])
                nc.sync.dma_start(xpv[:, c0:c0 + cw, :], dt[:, :cw, :])
                nc.vector.tensor_copy(xsv[:, 2 * c0:2 * (c0 + cw):2, :],
                                      tv[:, :cw, 0:DM])
                nc.vector.tensor_copy(xsv[:, 2 * c0 + 1:2 * (c0 + cw):2, :],
                                      tv[:, :cw, D:D + DM])

            # ---- phase D: software-pipelined main loop ---------------
            tiles = {}

            def produce(t_i):
                isl = slice(t_i * P, (t_i + 1) * P)
                osl = slice(t_i * K * TW, (t_i + 1) * K * TW)
                oh = ohp.tile([P, K, TW], fp8, tag="oh")
                nc.sync.dma_start(oh.rearrange("p k w -> p (k w)"),
                                  oh_ap[:, osl])
                ps = psp.tile([P, TW], f32, tag="ps")
                for ch in range(KC):
                    g = t_i * KC + ch
                    w = int(w_lo[g])
                    cs = slice(ch * P, (ch + 1) * P)
                    for k in range(K):
                        nc.tensor.matmul(
                            ps[:, cs], oh[:, k:k + 1, cs],
                            xsv[:, w + k, :],
                            start=(k == 0), stop=(k == K - 1))
                pt = pairp.tile([P, KC, 512], fp8, tag="pair")
                nc.gpsimd.dma_gather(pt[:], xp[:], idx_sb[:, isl],
                                     TW, TW, 512, single_packet=False)
                tiles[t_i] = (ps, pt)

            def consume(t_i):
                ps, pt = tiles.pop(t_i)
                slc = slice(t_i * KC, (t_i + 1) * KC)
                sl2 = slice(t_i * KC * 2, (t_i + 1) * KC * 2)
                pt16 = pt.bitcast(fp16)
                ssb = ssbp.tile([P, TW], fp16, tag="ssb")
                nc.scalar.activation(ssb[:], ps[:],
                                     mybir.ActivationFunctionType.Copy)
                sv = ssb.rearrange("p (c f) -> p c f", f=DM)
                de = dqp.tile([P, KC, DM], fp16, tag="dq")
                nc.vector.tensor_tensor(out=de[:], in0=sv[:],
                                        in1=pt[:, :, 0:DM],
                                        op=mybir.AluOpType.subtract)
                se = sqp.tile([P, KC, DM], fp16, tag="sq")
                nc.scalar.activation(se[:], de[:],
                                     mybir.ActivationFunctionType.Square)
                with nc.allow_low_precision("r2 fp16 accum: ~1% worst-case"):
                    nc.vector.tensor_reduce(r2e_w[:, slc], se[:],
                                            axis=mybir.AxisListType.X,
                                            op=mybir.AluOpType.add)
                do = dqp.tile([P, KC, DM], fp16, tag="dq")
                nc.vector.tensor_tensor(out=do[:], in0=sv[:],
                                        in1=pt[:, :, 256:256 + DM],
                                        op=mybir.AluOpType.subtract)
                so = sqp.tile([P, KC, DM], fp16, tag="sq")
                nc.scalar.activation(so[:], do[:],
                                     mybir.ActivationFunctionType.Square)
                with nc.allow_low_precision("r2 fp16 accum: ~1% worst-case"):
                    nc.vector.tensor_reduce(r2o_w[:, slc], so[:],
                                            axis=mybir.AxisListType.X,
                                            op=mybir.AluOpType.add)
                nc.scalar.activation(
                    m2_w[:, sl2].rearrange("p (c two) -> p c two", two=2),
                    pt16[:, :, 64:66], mybir.ActivationFunctionType.Copy)

            ntl = ntiles
            for t_i in range(ntl + 1):
                if t_i < ntl:
                    produce(t_i)
                if t_i > 0:
                    consume(t_i - 1)

            # ---- phase E: epilogue -----------------------------------
            # r2 = r2e + par*(r2o - r2e); m likewise; out = m - l*ln(r2+eps)
            nc.vector.tensor_tensor(out=r2o_w[:], in0=r2o_w[:], in1=r2e_w[:],
                                    op=mybir.AluOpType.subtract)
            nc.vector.tensor_tensor(out=r2o_w[:], in0=r2o_w[:], in1=par_sb[:],
                                    op=mybir.AluOpType.mult)
            nc.vector.tensor_tensor(out=r2e_w[:], in0=r2e_w[:], in1=r2o_w[:],
                                    op=mybir.AluOpType.add)
            nc.vector.tensor_tensor(out=tm_w.unsqueeze(2), in0=m2v[:, :, 1:2],
                                    in1=m2v[:, :, 0:1],
                                    op=mybir.AluOpType.subtract)
            nc.vector.tensor_tensor(out=tm_w[:], in0=tm_w[:], in1=par_sb[:],
                                    op=mybir.AluOpType.mult)
            nc.vector.tensor_tensor(out=mm_w.unsqueeze(2), in0=m2v[:, :, 0:1],
                                    in1=tm_w.unsqueeze(2),
                                    op=mybir.AluOpType.add)
            nc.scalar.activation(r2o_w[:], r2e_w[:],
                                 mybir.ActivationFunctionType.Ln,
                                 bias=cst[:, 1:2])
            nc.vector.scalar_tensor_tensor(
                out=outw[:], in0=r2o_w[:], scalar=cst[:, 0:1], in1=mm_w[:],
                op0=mybir.AluOpType.mult, op1=mybir.AluOpType.add)
            nc.sync.dma_start(out_ap[:], outw[:])

    nc.compile()
    return nc


_compiled = {}


def _get_compiled(C=C_DEF):
    if C not in _compiled:
        _compiled[C] = build_program(C)
    return _compiled[C]


def _pack_core(src, C):
    """Window-pack edges (sorted by src block) into C chunks of 128 slots.

    Returns slot2edge [C*128] int64 (-1 = pad). Raises OverflowError if C
    is too small.
    """
    w_lo = _w_lo(C)
    order = np.argsort(src % NB, kind="stable")
    blocks = (src[order] % NB).astype(np.int64)
    counts = np.bincount(blocks, minlength=NB)
    slot2edge = np.full(C * P, -1, np.int64)
    c = 0
    fill = 0
    pos = 0
    for b in range(NB):
        n = int(counts[b])
        while n > 0:
            while c < C and not (w_lo[c] <= b < w_lo[c] + K):
                c += 1
                fill = 0
            if c >= C:
                raise OverflowError(C)
            take = min(n, P - fill)
            s0 = c * P + fill
            slot2edge[s0:s0 + take] = order[pos:pos + take]
            pos += take
            n -= take
            fill += take
            if fill == P:
                c += 1
                fill = 0
                if c >= C and n > 0:
                    raise OverflowError(C)
    return slot2edge


def make_in_maps(x, edge_label_index, l_param, C=C_DEF):
    x = np.asarray(x, dtype=np.float32)
    x_pad = np.zeros((NPAD, D), np.float32)
    x_pad[:N] = x
    eli = np.asarray(edge_label_index)
    l = float(np.asarray(l_param).reshape(-1)[0])
    cstv = np.zeros((P, 2), np.float16)
    cstv[:, 0] = -l
    cstv[:, 1] = EPS
    w_lo = _w_lo(C)
    ntiles = C // KC
    chunk_of_slot = np.arange(C * P) // P

    in_maps = []
    slot_maps = []
    for core in range(NUM_CORES):
        sl = slice(core * EC, (core + 1) * EC)
        src = eli[0][sl].astype(np.int64)
        dst = eli[1][sl].astype(np.int64)
        s2e = _pack_core(src, C)
        real = s2e >= 0
        e = s2e[real]
        slots = np.arange(C * P)[real]
        vp = src[e] // NB                            # position in block
        vk = src[e] % NB - w_lo[chunk_of_slot[real]]  # [0, K)
        oh8 = np.zeros((P, ntiles, K, TW), np.uint8)
        oh8[vp, slots // TW, vk, slots % TW] = 0x38  # 1.0 in e4m3
        dstv = np.zeros(C * P, np.int16)
        dstv[real] = (dst[e] // 2).astype(np.int16)
        d16 = np.tile(dstv.reshape(C * P // 16, 16).T, (8, 1))
        par = np.zeros(C * P, np.float16)
        par[real] = (dst[e] % 2).astype(np.float16)
        in_maps.append({
            "xpad": x_pad,
            "dst16": np.ascontiguousarray(d16),
            "ohd": oh8.reshape(P, ntiles * K * TW).view(FP8NP),
            "par": np.ascontiguousarray(par.reshape(C, P).T),
            "cst": cstv,
        })
        slot_maps.append(s2e)
    return in_maps, slot_maps, C


def _unshard(results, slot_maps, C):
    out = np.empty(E, np.float32)
    for core in range(NUM_CORES):
        dev = results[core]["out"]          # [128, C]
        vals = dev.T.ravel()                # slot-major
        s2e = slot_maps[core]
        real = s2e >= 0
        core_out = np.empty(EC, np.float32)
        core_out[s2e[real]] = vals[real]
        out[core * EC:(core + 1) * EC] = core_out
    return out.reshape(E, 1)


def kernel(x, edge_label_index, l_param):
    C = C_DEF
    while True:
        try:
            in_maps, slot_maps, C = make_in_maps(
                x, edge_label_index, l_param, C)
            break
        except OverflowError:
            C += 2 * KC
    nc = _get_compiled(C)
    res = run_bass_kernel_spmd(nc, in_maps, list(range(NUM_CORES)))
    return _unshard(res.results, slot_maps, C)


# revision 14
# speedup vs baseline: 2.0517x; 1.0648x over previous
"""Trainium2 Bass kernel for nn_DecoderGravity (edge-list gravity decoder).

Computes, for each edge e with src s=idx[0,e], dst d=idx[1,e]:
    out[e] = x[d, 128] - l * log(sum_k (x[s,k]-x[d,k])^2 + 0.01)

The v1 kernel was bottlenecked by GPSIMD SWDGE descriptor generation for
dma_gather (~8ns/index, 160k indices/core = 1.3ms). This version halves
the descriptor stream and overlaps everything else under it:

  * dst side: ONE dma_gather stream from a "pair table" xp[25088, 512B]
    (fp8 features + fp16 mass for nodes 2r and 2r+1 in one row; index =
    dst//2 fits int16 without lo/hi bucketing). 86016 padded slots ->
    ~690us of Pool time, the kernel's roofline. Even/odd halves are
    blended at the r2/mass level with a host parity mask.
  * src side: NO dma_gather. Edges are sorted by src block (128 nodes)
    and packed into 128-edge chunks such that chunk c only draws from a
    static window of K=2 blocks. Host ships per-chunk one-hot planes
    (fp8; pure index metadata); the Tensor engine multiplies them with
    the fp8 node table x_sb [128, 391*128] to materialize gathered src
    rows in PSUM (edge-major), overlapped under the Pool roof.
  * r2 = reduce((s-d)^2): ACT copies PSUM->fp16, DVE subtracts, ACT
    squares, DVE reduces (fp16 accum; ~1% r2 error is far inside the
    gate) for both dst halves. Epilogue: out = m - l*ln(r2 + eps).
  * Emission is software-pipelined: produce(t) [oh DMA, matmuls,
    gather] is emitted before consume(t-1) so no engine head-of-line
    blocks the Pool descriptor stream.
"""

import numpy as np
import ml_dtypes

import concourse.bass as bass
import concourse.tile as tile
from concourse import bacc, mybir
from concourse.bass_utils import run_bass_kernel_spmd

# Problem constants (hardcoded per contract).
N = 50000
D = 129
DM = 128
E = 640000
NUM_CORES = 8
P = 128
EC = E // NUM_CORES          # 80000 edges per core
NPAD = 50176                 # N padded to 128*392
NPAIR = NPAD // 2            # pair-table rows
NB = 392                     # src blocks: node n in block n%392, row n//392
K = 2                        # block window size per chunk
C_DEF = 656                  # chunks of 128 edge slots (pad >= 2.5%)
KC = 16                      # chunks per gather tile (2048 slots)
TW = KC * P                  # slots per tile
EPS = 0.01

f32 = mybir.dt.float32
fp16 = mybir.dt.float16
fp8 = mybir.dt.float8e4
i16 = mybir.dt.int16
FP8NP = ml_dtypes.float8_e4m3


def _w_lo(C):
    alpha = NB / C
    return np.minimum((np.arange(C) * alpha).astype(int), NB - K)


def build_program(C=C_DEF):
    assert C % KC == 0
    ntiles = C // KC
    w_lo = _w_lo(C)
    nc = bacc.Bacc("TRN2", target_bir_lowering=False, debug=False,
                   num_devices=NUM_CORES)
    x_ap = nc.dram_tensor("xpad", [NPAD, D], f32, kind="ExternalInput").ap()
    d16_ap = nc.dram_tensor("dst16", [P, C * 8], i16,
                            kind="ExternalInput").ap()
    oh_ap = nc.dram_tensor("ohd", [P, ntiles * K * TW], fp8,
                           kind="ExternalInput").ap()
    par_ap = nc.dram_tensor("par", [P, C], fp16, kind="ExternalInput").ap()
    cst_ap = nc.dram_tensor("cst", [P, 2], fp16, kind="ExternalInput").ap()
    out_ap = nc.dram_tensor("out", [P, C], f32, kind="ExternalOutput").ap()

    xp = nc.dram_tensor("xp", [NPAIR, 512], fp8).ap()

    with tile.TileContext(nc) as tc:
        with (
            tc.tile_pool(name="xt", bufs=2) as xtp,
            tc.tile_pool(name="pair", bufs=2) as pairp,
            tc.tile_pool(name="xsb", bufs=1) as xsbp,
            tc.tile_pool(name="oh", bufs=2) as ohp,
            tc.tile_pool(name="ssb", bufs=2) as ssbp,
            tc.tile_pool(name="dq", bufs=2) as dqp,
            tc.tile_pool(name="sq", bufs=2) as sqp,
            tc.tile_pool(name="wide", bufs=1) as widep,
            tc.tile_pool(name="ps", bufs=2, space="PSUM") as psp,
        ):
            # ---- phase C: small loads (first: gathers dep on idx) ----
            idx_sb = widep.tile([P, C * 8], i16, tag="idx")
            nc.sync.dma_start(idx_sb[:], d16_ap[:])
            par_sb = widep.tile([P, C], fp16, tag="par")
            nc.sync.dma_start(par_sb[:], par_ap[:])
            cst = widep.tile([P, 2], fp16, tag="cst")
            nc.sync.dma_start(cst[:], cst_ap[:])

            r2e_w = widep.tile([P, C], fp16, tag="r2e")
            r2o_w = widep.tile([P, C], fp16, tag="r2o")
            m2_w = widep.tile([P, C * 2], fp16, tag="m2")
            tm_w = widep.tile([P, C], fp16, tag="tm")
            mm_w = widep.tile([P, C], fp16, tag="mm")
            outw = widep.tile([P, C], f32, tag="outw")
            m2v = m2_w.rearrange("p (c two) -> p c two", two=2)

            # ---- phase A: build pair table xp + x_sb in one x pass ----
            # x rows viewed as [p, 196 pairs, 2, 129]; xp as [p, 196, 512].
            # Node n = p*392 + c2 (c2 = 2c+t): src block = n%392 = c2,
            # position in block = n//392 = p. So the same tiles also fill
            # x_sb (strided over c2 parity) with no second x read.
            xv = x_ap[:].rearrange("(p c two) d -> p c (two d)", p=P, two=2)
            xpv = xp[:].rearrange("(p c) d -> p c d", p=P)
            x_sb = xsbp.tile([P, NB * DM], fp8, tag="xsb")
            xsv = x_sb.rearrange("p (c f) -> p c f", f=DM)
            CP = NPAIR // P  # 196
            for c0 in range(0, CP, 16):
                cw = min(16, CP - c0)
                t = xtp.tile([P, 4128], f32, tag="xt")
                tv = t.rearrange("p (c d) -> p c d", d=2 * D)
                nc.sync.dma_start(tv[:, :cw, :], xv[:, c0:c0 + cw, :])
                dt = pairp.tile([P, KC, 512], fp8, tag="pair")
                dt16 = dt.bitcast(fp16)
                nc.vector.tensor_copy(dt[:, :cw, 0:DM], tv[:, :cw, 0:DM])
                nc.vector.tensor_copy(dt16[:, :cw, 64:65],
                                      tv[:, :cw, DM:DM + 1])
                nc.vector.tensor_copy(dt[:, :cw, 256:256 + DM],
                                      tv[:, :cw, D:D + DM])
                nc.vector.tensor_copy(dt16[:, :cw, 65:66],
                                      tv[:, :cw, 2 * D - 1:2 * D])
                nc.sync.dma_start(xpv[:, c0:c0 + cw, :], dt[:, :cw, :])
                nc.vector.tensor_copy(xsv[:, 2 * c0:2 * (c0 + cw):2, :],
                                      tv[:, :cw, 0:DM])
                nc.vector.tensor_copy(xsv[:, 2 * c0 + 1:2 * (c0 + cw):2, :],
                                      tv[:, :cw, D:D + DM])

            # ---- phase D: software-pipelined main loop ---------------
            # gathers are paired (4096 idx) to amortize SWDGE overhead
            tiles = {}
            gtiles = {}

            def produce(t_i):
                if t_i % 2 == 0:
                    nt2 = min(2, ntiles - t_i)
                    isl = slice(t_i * P, (t_i + nt2) * P)
                    gt = pairp.tile([P, 2 * KC, 512], fp8, tag="gpair")
                    nc.gpsimd.dma_gather(gt[:, :nt2 * KC, :], xp[:],
                                         idx_sb[:, isl], nt2 * TW, nt2 * TW,
                                         512, single_packet=False)
                    gtiles[t_i // 2] = gt
                osl = slice(t_i * K * TW, (t_i + 1) * K * TW)
                oh = ohp.tile([P, K, TW], fp8, tag="oh")
                nc.sync.dma_start(oh.rearrange("p k w -> p (k w)"),
                                  oh_ap[:, osl])
                ps = psp.tile([P, TW], f32, tag="ps")
                for ch in range(KC):
                    g = t_i * KC + ch
                    w = int(w_lo[g])
                    cs = slice(ch * P, (ch + 1) * P)
                    for k in range(K):
                        nc.tensor.matmul(
                            ps[:, cs], oh[:, k:k + 1, cs],
                            xsv[:, w + k, :],
                            start=(k == 0), stop=(k == K - 1))
                tiles[t_i] = ps

            def consume(t_i):
                ps = tiles.pop(t_i)
                gt = gtiles[t_i // 2]
                h0 = (t_i % 2) * KC
                pt = gt[:, h0:h0 + KC, :]
                slc = slice(t_i * KC, (t_i + 1) * KC)
                sl2 = slice(t_i * KC * 2, (t_i + 1) * KC * 2)
                pt16 = gt.bitcast(fp16)[:, h0:h0 + KC, :]
                ssb = ssbp.tile([P, TW], fp16, tag="ssb")
                nc.scalar.activation(ssb[:], ps[:],
                                     mybir.ActivationFunctionType.Copy)
                sv = ssb.rearrange("p (c f) -> p c f", f=DM)
                de = dqp.tile([P, KC, DM], fp16, tag="dq")
                nc.vector.tensor_tensor(out=de[:], in0=sv[:],
                                        in1=pt[:, :, 0:DM],
                                        op=mybir.AluOpType.subtract)
                se = sqp.tile([P, KC, DM], fp16, tag="sq")
                nc.scalar.activation(se[:], de[:],
                                     mybir.ActivationFunctionType.Square)
                with nc.allow_low_precision("r2 fp16 accum: ~1% worst-case"):
                    nc.vector.tensor_reduce(r2e_w[:, slc], se[:],
                                            axis=mybir.AxisListType.X,
                                            op=mybir.AluOpType.add)
                do = dqp.tile([P, KC, DM], fp16, tag="dq")
                nc.vector.tensor_tensor(out=do[:], in0=sv[:],
                                        in1=pt[:, :, 256:256 + DM],
                                        op=mybir.AluOpType.subtract)
                so = sqp.tile([P, KC, DM], fp16, tag="sq")
                nc.scalar.activation(so[:], do[:],
                                     mybir.ActivationFunctionType.Square)
                with nc.allow_low_precision("r2 fp16 accum: ~1% worst-case"):
                    nc.vector.tensor_reduce(r2o_w[:, slc], so[:],
                                            axis=mybir.AxisListType.X,
                                            op=mybir.AluOpType.add)
                nc.scalar.activation(
                    m2_w[:, sl2].rearrange("p (c two) -> p c two", two=2),
                    pt16[:, :, 64:66], mybir.ActivationFunctionType.Copy)

            ntl = ntiles
            for t_i in range(ntl + 1):
                if t_i < ntl:
                    produce(t_i)
                if t_i > 0:
                    consume(t_i - 1)

            # ---- phase E: epilogue -----------------------------------
            # r2 = r2e + par*(r2o - r2e); m likewise; out = m - l*ln(r2+eps)
            nc.vector.tensor_tensor(out=r2o_w[:], in0=r2o_w[:], in1=r2e_w[:],
                                    op=mybir.AluOpType.subtract)
            nc.vector.tensor_tensor(out=r2o_w[:], in0=r2o_w[:], in1=par_sb[:],
                                    op=mybir.AluOpType.mult)
            nc.vector.tensor_tensor(out=r2e_w[:], in0=r2e_w[:], in1=r2o_w[:],
                                    op=mybir.AluOpType.add)
            nc.vector.tensor_tensor(out=tm_w.unsqueeze(2), in0=m2v[:, :, 1:2],
                                    in1=m2v[:, :, 0:1],
                                    op=mybir.AluOpType.subtract)
            nc.vector.tensor_tensor(out=tm_w[:], in0=tm_w[:], in1=par_sb[:],
                                    op=mybir.AluOpType.mult)
            nc.vector.tensor_tensor(out=mm_w.unsqueeze(2), in0=m2v[:, :, 0:1],
                                    in1=tm_w.unsqueeze(2),
                                    op=mybir.AluOpType.add)
            nc.scalar.activation(r2o_w[:], r2e_w[:],
                                 mybir.ActivationFunctionType.Ln,
                                 bias=cst[:, 1:2])
            nc.vector.scalar_tensor_tensor(
                out=outw[:], in0=r2o_w[:], scalar=cst[:, 0:1], in1=mm_w[:],
                op0=mybir.AluOpType.mult, op1=mybir.AluOpType.add)
            nc.sync.dma_start(out_ap[:], outw[:])

    nc.compile()
    return nc


_compiled = {}


def _get_compiled(C=C_DEF):
    if C not in _compiled:
        _compiled[C] = build_program(C)
    return _compiled[C]


def _pack_core(src, C):
    """Window-pack edges (sorted by src block) into C chunks of 128 slots.

    Returns slot2edge [C*128] int64 (-1 = pad). Raises OverflowError if C
    is too small.
    """
    w_lo = _w_lo(C)
    order = np.argsort(src % NB, kind="stable")
    blocks = (src[order] % NB).astype(np.int64)
    counts = np.bincount(blocks, minlength=NB)
    slot2edge = np.full(C * P, -1, np.int64)
    c = 0
    fill = 0
    pos = 0
    for b in range(NB):
        n = int(counts[b])
        while n > 0:
            while c < C and not (w_lo[c] <= b < w_lo[c] + K):
                c += 1
                fill = 0
            if c >= C:
                raise OverflowError(C)
            take = min(n, P - fill)
            s0 = c * P + fill
            slot2edge[s0:s0 + take] = order[pos:pos + take]
            pos += take
            n -= take
            fill += take
            if fill == P:
                c += 1
                fill = 0
                if c >= C and n > 0:
                    raise OverflowError(C)
    return slot2edge


def make_in_maps(x, edge_label_index, l_param, C=C_DEF):
    x = np.asarray(x, dtype=np.float32)
    x_pad = np.zeros((NPAD, D), np.float32)
    x_pad[:N] = x
    eli = np.asarray(edge_label_index)
    l = float(np.asarray(l_param).reshape(-1)[0])
    cstv = np.zeros((P, 2), np.float16)
    cstv[:, 0] = -l
    cstv[:, 1] = EPS
    w_lo = _w_lo(C)
    ntiles = C // KC
    chunk_of_slot = np.arange(C * P) // P

    in_maps = []
    slot_maps = []
    for core in range(NUM_CORES):
        sl = slice(core * EC, (core + 1) * EC)
        src = eli[0][sl].astype(np.int64)
        dst = eli[1][sl].astype(np.int64)
        s2e = _pack_core(src, C)
        real = s2e >= 0
        e = s2e[real]
        slots = np.arange(C * P)[real]
        vp = src[e] // NB                            # position in block
        vk = src[e] % NB - w_lo[chunk_of_slot[real]]  # [0, K)
        oh8 = np.zeros((P, ntiles, K, TW), np.uint8)
        oh8[vp, slots // TW, vk, slots % TW] = 0x38  # 1.0 in e4m3
        dstv = np.zeros(C * P, np.int16)
        dstv[real] = (dst[e] // 2).astype(np.int16)
        d16 = np.tile(dstv.reshape(C * P // 16, 16).T, (8, 1))
        par = np.zeros(C * P, np.float16)
        par[real] = (dst[e] % 2).astype(np.float16)
        in_maps.append({
            "xpad": x_pad,
            "dst16": np.ascontiguousarray(d16),
            "ohd": oh8.reshape(P, ntiles * K * TW).view(FP8NP),
            "par": np.ascontiguousarray(par.reshape(C, P).T),
            "cst": cstv,
        })
        slot_maps.append(s2e)
    return in_maps, slot_maps, C


def _unshard(results, slot_maps, C):
    out = np.empty(E, np.float32)
    for core in range(NUM_CORES):
        dev = results[core]["out"]          # [128, C]
        vals = dev.T.ravel()                # slot-major
        s2e = slot_maps[core]
        real = s2e >= 0
        core_out = np.empty(EC, np.float32)
        core_out[s2e[real]] = vals[real]
        out[core * EC:(core + 1) * EC] = core_out
    return out.reshape(E, 1)


def kernel(x, edge_label_index, l_param):
    C = C_DEF
    while True:
        try:
            in_maps, slot_maps, C = make_in_maps(
                x, edge_label_index, l_param, C)
            break
        except OverflowError:
            C += 2 * KC
    nc = _get_compiled(C)
    res = run_bass_kernel_spmd(nc, in_maps, list(range(NUM_CORES)))
    return _unshard(res.results, slot_maps, C)


# revision 16
# speedup vs baseline: 2.1199x; 1.0333x over previous
"""Trainium2 Bass kernel for nn_DecoderGravity (edge-list gravity decoder).

Computes, for each edge e with src s=idx[0,e], dst d=idx[1,e]:
    out[e] = x[d, 128] - l * log(sum_k (x[s,k]-x[d,k])^2 + 0.01)

The v1 kernel was bottlenecked by GPSIMD SWDGE descriptor generation for
dma_gather (~8ns/index, 160k indices/core = 1.3ms). This version halves
the descriptor stream and overlaps everything else under it:

  * dst side: ONE dma_gather stream from a "pair table" xp[25088, 512B]
    (fp8 features + fp16 mass for nodes 2r and 2r+1 in one row; index =
    dst//2 fits int16 without lo/hi bucketing). 86016 padded slots ->
    ~690us of Pool time, the kernel's roofline. Even/odd halves are
    blended at the r2/mass level with a host parity mask.
  * src side: NO dma_gather. Edges are sorted by src block (128 nodes)
    and packed into 128-edge chunks such that chunk c only draws from a
    static window of K=2 blocks. Host ships per-chunk one-hot planes
    (fp8; pure index metadata); the Tensor engine multiplies them with
    the fp8 node table x_sb [128, 391*128] to materialize gathered src
    rows in PSUM (edge-major), overlapped under the Pool roof.
  * r2 = reduce((s-d)^2): ACT copies PSUM->fp16, DVE subtracts, ACT
    squares, DVE reduces (fp16 accum; ~1% r2 error is far inside the
    gate) for both dst halves. Epilogue: out = m - l*ln(r2 + eps).
  * Emission is software-pipelined: produce(t) [oh DMA, matmuls,
    gather] is emitted before consume(t-1) so no engine head-of-line
    blocks the Pool descriptor stream.
"""

import numpy as np
import ml_dtypes

import concourse.bass as bass
import concourse.tile as tile
from concourse import bacc, mybir
from concourse.bass_utils import run_bass_kernel_spmd

# Problem constants (hardcoded per contract).
N = 50000
D = 129
DM = 128
E = 640000
NUM_CORES = 8
P = 128
EC = E // NUM_CORES          # 80000 edges per core
NPAD = 50176                 # N padded to 128*392
NPAIR = NPAD // 2            # pair-table rows
NB = 392                     # src blocks: node n in block n%392, row n//392
K = 2                        # block window size per chunk
C_DEF = 656                  # chunks of 128 edge slots (pad >= 2.5%)
KC = 16                      # chunks per gather tile (2048 slots)
TW = KC * P                  # slots per tile
EPS = 0.01

f32 = mybir.dt.float32
fp16 = mybir.dt.float16
fp8 = mybir.dt.float8e4
i16 = mybir.dt.int16
FP8NP = ml_dtypes.float8_e4m3


def _w_lo(C):
    alpha = NB / C
    return np.minimum((np.arange(C) * alpha).astype(int), NB - K)


def build_program(C=C_DEF):
    assert C % KC == 0
    ntiles = C // KC
    w_lo = _w_lo(C)
    nc = bacc.Bacc("TRN2", target_bir_lowering=False, debug=False,
                   num_devices=NUM_CORES)
    x_ap = nc.dram_tensor("xpad", [NPAD, D], f32, kind="ExternalInput").ap()
    d16_ap = nc.dram_tensor("dst16", [P, C * 8], i16,
                            kind="ExternalInput").ap()
    oh_ap = nc.dram_tensor("ohd", [P, ntiles * K * TW], fp8,
                           kind="ExternalInput").ap()
    par_ap = nc.dram_tensor("par", [P, C], fp16, kind="ExternalInput").ap()
    cst_ap = nc.dram_tensor("cst", [P, 2], fp16, kind="ExternalInput").ap()
    out_ap = nc.dram_tensor("out", [P, C], f32, kind="ExternalOutput").ap()

    xp = nc.dram_tensor("xp", [NPAIR, 512], fp8).ap()

    with tile.TileContext(nc) as tc:
        with (
            tc.tile_pool(name="xt", bufs=3) as xtp,
            tc.tile_pool(name="pair", bufs=2) as pairp,
            tc.tile_pool(name="xsb", bufs=1) as xsbp,
            tc.tile_pool(name="oh", bufs=2) as ohp,
            tc.tile_pool(name="ssb", bufs=2) as ssbp,
            tc.tile_pool(name="dq", bufs=2) as dqp,
            tc.tile_pool(name="sq", bufs=2) as sqp,
            tc.tile_pool(name="wide", bufs=1) as widep,
            tc.tile_pool(name="ps", bufs=2, space="PSUM") as psp,
        ):
            # ---- phase C: small loads (first: gathers dep on idx) ----
            idx_sb = widep.tile([P, C * 8], i16, tag="idx")
            nc.sync.dma_start(idx_sb[:], d16_ap[:])
            par_sb = widep.tile([P, C], fp16, tag="par")
            nc.sync.dma_start(par_sb[:], par_ap[:])
            cst = widep.tile([P, 2], fp16, tag="cst")
            nc.sync.dma_start(cst[:], cst_ap[:])

            r2e_w = widep.tile([P, C], fp16, tag="r2e")
            r2o_w = widep.tile([P, C], fp16, tag="r2o")
            m2_w = widep.tile([P, C * 2], fp16, tag="m2")
            tm_w = widep.tile([P, C], fp16, tag="tm")
            mm_w = widep.tile([P, C], fp16, tag="mm")
            outw = widep.tile([P, C], f32, tag="outw")
            m2v = m2_w.rearrange("p (c two) -> p c two", two=2)

            # ---- phase A: build pair table xp + x_sb in one x pass ----
            # x rows viewed as [p, 196 pairs, 2, 129]; xp as [p, 196, 512].
            # Node n = p*392 + c2 (c2 = 2c+t): src block = n%392 = c2,
            # position in block = n//392 = p. So the same tiles also fill
            # x_sb (strided over c2 parity) with no second x read.
            xv = x_ap[:].rearrange("(p c two) d -> p c (two d)", p=P, two=2)
            xpv = xp[:].rearrange("(p c) d -> p c d", p=P)
            x_sb = xsbp.tile([P, NB * DM], fp8, tag="xsb")
            xsv = x_sb.rearrange("p (c f) -> p c f", f=DM)
            CP = NPAIR // P  # 196
            for c0 in range(0, CP, 16):
                cw = min(16, CP - c0)
                t = xtp.tile([P, 4128], f32, tag="xt")
                tv = t.rearrange("p (c d) -> p c d", d=2 * D)
                nc.sync.dma_start(tv[:, :cw, :], xv[:, c0:c0 + cw, :])
                dt = pairp.tile([P, KC, 512], fp8, tag="pair")
                dt16 = dt.bitcast(fp16)
                nc.vector.tensor_copy(dt[:, :cw, 0:DM], tv[:, :cw, 0:DM])
                nc.vector.tensor_copy(dt16[:, :cw, 64:65],
                                      tv[:, :cw, DM:DM + 1])
                nc.vector.tensor_copy(dt[:, :cw, 256:256 + DM],
                                      tv[:, :cw, D:D + DM])
                nc.vector.tensor_copy(dt16[:, :cw, 65:66],
                                      tv[:, :cw, 2 * D - 1:2 * D])
                nc.sync.dma_start(xpv[:, c0:c0 + cw, :], dt[:, :cw, :])
                nc.vector.tensor_copy(xsv[:, 2 * c0:2 * (c0 + cw):2, :],
                                      tv[:, :cw, 0:DM])
                nc.vector.tensor_copy(xsv[:, 2 * c0 + 1:2 * (c0 + cw):2, :],
                                      tv[:, :cw, D:D + DM])

            # ---- phase D: software-pipelined main loop ---------------
            # gathers are paired (4096 idx) to amortize SWDGE overhead
            tiles = {}
            gtiles = {}

            def produce(t_i):
                if t_i % 2 == 0:
                    nt2 = min(2, ntiles - t_i)
                    isl = slice(t_i * P, (t_i + nt2) * P)
                    gt = pairp.tile([P, 2 * KC, 512], fp8, tag="gpair")
                    nc.gpsimd.dma_gather(gt[:, :nt2 * KC, :], xp[:],
                                         idx_sb[:, isl], nt2 * TW, nt2 * TW,
                                         512, single_packet=False)
                    gtiles[t_i // 2] = gt
                osl = slice(t_i * K * TW, (t_i + 1) * K * TW)
                oh = ohp.tile([P, K, TW], fp8, tag="oh")
                nc.sync.dma_start(oh.rearrange("p k w -> p (k w)"),
                                  oh_ap[:, osl])
                ps = psp.tile([P, TW], f32, tag="ps")
                for ch in range(KC):
                    g = t_i * KC + ch
                    w = int(w_lo[g])
                    cs = slice(ch * P, (ch + 1) * P)
                    for k in range(K):
                        nc.tensor.matmul(
                            ps[:, cs], oh[:, k:k + 1, cs],
                            xsv[:, w + k, :],
                            start=(k == 0), stop=(k == K - 1))
                tiles[t_i] = ps

            def consume(t_i):
                ps = tiles.pop(t_i)
                gt = gtiles[t_i // 2]
                h0 = (t_i % 2) * KC
                pt = gt[:, h0:h0 + KC, :]
                slc = slice(t_i * KC, (t_i + 1) * KC)
                sl2 = slice(t_i * KC * 2, (t_i + 1) * KC * 2)
                pt16 = gt.bitcast(fp16)[:, h0:h0 + KC, :]
                ssb = ssbp.tile([P, TW], fp16, tag="ssb")
                nc.scalar.activation(ssb[:], ps[:],
                                     mybir.ActivationFunctionType.Copy)
                sv = ssb.rearrange("p (c f) -> p c f", f=DM)
                de = dqp.tile([P, KC, DM], fp16, tag="dq")
                nc.vector.tensor_tensor(out=de[:], in0=sv[:],
                                        in1=pt[:, :, 0:DM],
                                        op=mybir.AluOpType.subtract)
                se = sqp.tile([P, KC, DM], fp16, tag="sq")
                nc.scalar.activation(se[:], de[:],
                                     mybir.ActivationFunctionType.Square)
                with nc.allow_low_precision("r2 fp16 accum: ~1% worst-case"):
                    nc.vector.tensor_reduce(r2e_w[:, slc], se[:],
                                            axis=mybir.AxisListType.X,
                                            op=mybir.AluOpType.add)
                do = dqp.tile([P, KC, DM], fp16, tag="dq")
                nc.vector.tensor_tensor(out=do[:], in0=sv[:],
                                        in1=pt[:, :, 256:256 + DM],
                                        op=mybir.AluOpType.subtract)
                so = sqp.tile([P, KC, DM], fp16, tag="sq")
                nc.scalar.activation(so[:], do[:],
                                     mybir.ActivationFunctionType.Square)
                with nc.allow_low_precision("r2 fp16 accum: ~1% worst-case"):
                    nc.vector.tensor_reduce(r2o_w[:, slc], so[:],
                                            axis=mybir.AxisListType.X,
                                            op=mybir.AluOpType.add)
                nc.scalar.activation(
                    m2_w[:, sl2].rearrange("p (c two) -> p c two", two=2),
                    pt16[:, :, 64:66], mybir.ActivationFunctionType.Copy)

            # epilogue over a column range (split so most of it overlaps
            # the tail of the gather stream):
            # r2 = r2e + par*(r2o - r2e); m likewise; out = m - l*ln(r2+eps)
            def epilogue(c0, c1):
                cs = slice(c0, c1)
                cs2 = slice(c0, c1)
                nc.vector.tensor_tensor(
                    out=r2o_w[:, cs], in0=r2o_w[:, cs], in1=r2e_w[:, cs],
                    op=mybir.AluOpType.subtract)
                nc.vector.tensor_tensor(
                    out=r2o_w[:, cs], in0=r2o_w[:, cs], in1=par_sb[:, cs],
                    op=mybir.AluOpType.mult)
                nc.vector.tensor_tensor(
                    out=r2e_w[:, cs], in0=r2e_w[:, cs], in1=r2o_w[:, cs],
                    op=mybir.AluOpType.add)
                nc.vector.tensor_tensor(
                    out=tm_w[:, cs].unsqueeze(2), in0=m2v[:, cs2, 1:2],
                    in1=m2v[:, cs2, 0:1], op=mybir.AluOpType.subtract)
                nc.vector.tensor_tensor(
                    out=tm_w[:, cs], in0=tm_w[:, cs], in1=par_sb[:, cs],
                    op=mybir.AluOpType.mult)
                nc.vector.tensor_tensor(
                    out=mm_w[:, cs].unsqueeze(2), in0=m2v[:, cs2, 0:1],
                    in1=tm_w[:, cs].unsqueeze(2), op=mybir.AluOpType.add)
                nc.scalar.activation(r2o_w[:, cs], r2e_w[:, cs],
                                     mybir.ActivationFunctionType.Ln,
                                     bias=cst[:, 1:2])
                nc.vector.scalar_tensor_tensor(
                    out=outw[:, cs], in0=r2o_w[:, cs], scalar=cst[:, 0:1],
                    in1=mm_w[:, cs],
                    op0=mybir.AluOpType.mult, op1=mybir.AluOpType.add)
                nc.sync.dma_start(out_ap[:, cs], outw[:, cs])

            ntl = ntiles
            for t_i in range(ntl + 1):
                if t_i < ntl:
                    produce(t_i)
                if t_i > 0:
                    consume(t_i - 1)
                    if t_i == ntl - 1:
                        # all but the last tile's columns: overlaps the
                        # final gather + consume
                        epilogue(0, (ntl - 1) * KC)
            epilogue((ntl - 1) * KC, C)

    nc.compile()
    return nc


_compiled = {}


def _get_compiled(C=C_DEF):
    if C not in _compiled:
        _compiled[C] = build_program(C)
    return _compiled[C]


def _pack_core(src, C):
    """Window-pack edges (sorted by src block) into C chunks of 128 slots.

    Returns slot2edge [C*128] int64 (-1 = pad). Raises OverflowError if C
    is too small.
    """
    w_lo = _w_lo(C)
    order = np.argsort(src % NB, kind="stable")
    blocks = (src[order] % NB).astype(np.int64)
    counts = np.bincount(blocks, minlength=NB)
    slot2edge = np.full(C * P, -1, np.int64)
    c = 0
    fill = 0
    pos = 0
    for b in range(NB):
        n = int(counts[b])
        while n > 0:
            while c < C and not (w_lo[c] <= b < w_lo[c] + K):
                c += 1
                fill = 0
            if c >= C:
                raise OverflowError(C)
            take = min(n, P - fill)
            s0 = c * P + fill
            slot2edge[s0:s0 + take] = order[pos:pos + take]
            pos += take
            n -= take
            fill += take
            if fill == P:
                c += 1
                fill = 0
                if c >= C and n > 0:
                    raise OverflowError(C)
    return slot2edge


def make_in_maps(x, edge_label_index, l_param, C=C_DEF):
    x = np.asarray(x, dtype=np.float32)
    x_pad = np.zeros((NPAD, D), np.float32)
    x_pad[:N] = x
    eli = np.asarray(edge_label_index)
    l = float(np.asarray(l_param).reshape(-1)[0])
    cstv = np.zeros((P, 2), np.float16)
    cstv[:, 0] = -l
    cstv[:, 1] = EPS
    w_lo = _w_lo(C)
    ntiles = C // KC
    chunk_of_slot = np.arange(C * P) // P

    in_maps = []
    slot_maps = []
    for core in range(NUM_CORES):
        sl = slice(core * EC, (core + 1) * EC)
        src = eli[0][sl].astype(np.int64)
        dst = eli[1][sl].astype(np.int64)
        s2e = _pack_core(src, C)
        real = s2e >= 0
        e = s2e[real]
        slots = np.arange(C * P)[real]
        vp = src[e] // NB                            # position in block
        vk = src[e] % NB - w_lo[chunk_of_slot[real]]  # [0, K)
        oh8 = np.zeros((P, ntiles, K, TW), np.uint8)
        oh8[vp, slots // TW, vk, slots % TW] = 0x38  # 1.0 in e4m3
        dstv = np.zeros(C * P, np.int16)
        dstv[real] = (dst[e] // 2).astype(np.int16)
        d16 = np.tile(dstv.reshape(C * P // 16, 16).T, (8, 1))
        par = np.zeros(C * P, np.float16)
        par[real] = (dst[e] % 2).astype(np.float16)
        in_maps.append({
            "xpad": x_pad,
            "dst16": np.ascontiguousarray(d16),
            "ohd": oh8.reshape(P, ntiles * K * TW).view(FP8NP),
            "par": np.ascontiguousarray(par.reshape(C, P).T),
            "cst": cstv,
        })
        slot_maps.append(s2e)
    return in_maps, slot_maps, C


def _unshard(results, slot_maps, C):
    out = np.empty(E, np.float32)
    for core in range(NUM_CORES):
        dev = results[core]["out"]          # [128, C]
        vals = dev.T.ravel()                # slot-major
        s2e = slot_maps[core]
        real = s2e >= 0
        core_out = np.empty(EC, np.float32)
        core_out[s2e[real]] = vals[real]
        out[core * EC:(core + 1) * EC] = core_out
    return out.reshape(E, 1)


def kernel(x, edge_label_index, l_param):
    C = C_DEF
    while True:
        try:
            in_maps, slot_maps, C = make_in_maps(
                x, edge_label_index, l_param, C)
            break
        except OverflowError:
            C += 2 * KC
    nc = _get_compiled(C)
    res = run_bass_kernel_spmd(nc, in_maps, list(range(NUM_CORES)))
    return _unshard(res.results, slot_maps, C)
